# revision 18
# baseline (speedup 1.0000x reference)
"""Trainium2 Bass kernel for nn_Attention: fused QKV + RoPE + softmax attention + o_proj.

Sharding (8 cores): core c -> (batch b = c//2, head-half g = c%2).
Each core computes 8 of 16 heads for one batch; host sums the two
head-half partials per batch and transposes.

Design (driven by the TimelineSim cost model):
  - ACT exp is the binding engine: 256 blocks x [128, 2, 512] = 265.7us
    busy.  Everything else is scheduled to keep the exp stream gapless:
    scores lead exp by 2 blocks through a 2-tile psum ring; projections,
    P@V, and o_proj fill the remaining PE bandwidth.
  - Projections (QKV) run as fp8e4m3 hi/lo DoubleRow matmuls: w is
    scaled 32x on the host and split into hi+lo fp8, h likewise; the
    three products w_hi*h_hi + w_lo*h_hi + w_hi*h_lo reconstruct the
    bf16-accurate result at 0.75x the PE cost of fp16 matmuls
    (DoubleRow contracts 2 k-tiles per pass at 0.5 cycles/row).  The
    1/32 descale folds into the cos/sin tables (q/k) and the V psum
    copy.
  - Background work (K/Q/V projections, o_proj) is placed by a greedy
    earliest-deadline-first scheduler with a per-block PE cycle budget
    equal to the ACT exp cadence (2491 cycles/block), so PE never jams
    locally and never starves the exp stream.  Items are emitted in
    1024-cycle phases; an open item's phases complete before the next
    item starts (protects the 2-buf psum ring).
  - 'stag' unit order staggers both each pair's K/V deadline and each
    chunk's o_proj release across the run.
  - P@V with E stationary: per (head, q-block) one psum group
    [128 q, 65] over 16 kpos tiles (64 V columns + ones column ->
    denominator in column 64).  Normalize on DVE (reciprocal +
    tensor_scalar), SBUF->SBUF DMA transpose assembles attnT.
  - DMA: per-chunk contiguous layouts (chunk-major hT/cossin,
    feature-major wqk/wv) keep every transfer >= 1KB/descriptor; the
    prologue order is tuned so the first exp fires at ~11us.
PSUM banks: scores 2x2 + P@V 2 + projection ping-pong 2 = 8.
"""
import os
import sys

sys.path.insert(0, "/opt/trn_rl_repo")

import heapq
import numpy as np
import ml_dtypes

import concourse.bass as bass
import concourse.mybir as mybir
import concourse.tile as tile
from concourse.bass_utils import run_bass_kernel_spmd
from concourse.vector_clock import ScopedClock, VectorClock

# ---------------------------------------------------------------------------
# Patch TileContext._drain_and_barrier: the walrus build in this container
# allows only ONE sync-wait per instruction; Tile's tail drain carries one
# wait per active proc.  Split them into single-wait NOPs on SP.
N_PROCS = 27


def _patched_drain_and_barrier(self, tick_clock, wait_clock):
    nc = self.nc
    gc = tick_clock.global_clock
    for p in range(N_PROCS):
        t = gc[p]
        if t > 0:
            nop = nc.sync.nop(nofuse=True)
            vc = VectorClock([t if q == p else 0 for q in range(N_PROCS)])
            wait_clock.add_sem_waits(nop.ins, ScopedClock({None: vc}))
    nc.sync.drain()
    nc.all_engine_barrier()
    assert self.sems is not None
    popped = nc._tile_sem_poison_stack.pop()
    assert popped is self._sem_poison
    nc.clear_and_free_semaphores(list(self.sems.allocated().values()))
    nc.all_engine_barrier()


tile.TileContext._drain_and_barrier = _patched_drain_and_barrier


def _split_excess_waits(nc):
    """walrus in this container accepts 1 sync-wait per instruction (2 on
    EventSemaphore).  Move excess waits onto EventSemaphore instructions
    inserted just before, on the same engine."""
    for f in nc.m.functions:
        for bb in f.blocks:
            new_insts = []
            changed = False
            for ins in bb.instructions:
                si = ins.sync_info
                waits = list(si.on_wait) if si is not None else []
                cap = 2 if isinstance(ins, mybir.InstEventSemaphore) else 1
                if len(waits) > cap:
                    changed = True
                    excess = waits[: len(waits) - cap]
                    for i in range(0, len(excess), 2):
                        ev = mybir.InstEventSemaphore(
                            name=f"I-{nc.next_id()}",
                            engine=ins.engine,
                            ins=[],
                            outs=[],
                            sync_info=mybir.SyncInfo(
                                on_wait=excess[i : i + 2], on_update=[]
                            ),
                        )
                        nc.register_instruction(ev)
                        new_insts.append(ev)
                    si.on_wait = waits[len(waits) - cap :]
                new_insts.append(ins)
            if changed:
                bb.instructions[:] = new_insts
# ---------------------------------------------------------------------------

B, S, H, NH, HD = 4, 2048, 1024, 16, 64
HPC = NH // 2          # heads per core
PAIRS = HPC // 2       # head pairs per core
HT = H // 128          # hidden-dim tiles
QKF = 2 * HPC * HD     # q+k features per core (1024)
VF = HPC * HD          # v features per core (512)
SC = 512               # seq chunk (psum bank pair)
NSC = S // SC
KT = S // 128          # kpos tiles
QB = SC // 128         # q blocks per chunk (4)
BF = mybir.dt.float16
F32 = mybir.dt.float32
E4 = mybir.dt.float8e4
E4NP = ml_dtypes.float8_e4m3
WSCALE = 32.0  # host upscale of w_qkv before fp8 split (denormal floor)
EXP_SCALE = 1.0 / float(np.sqrt(HD))

# Pipeline knobs
EPS_BUFS = 23          # e-tile ring; P@V groups must finish before exp reuses bufs
PV_PER_BLOCK = int(os.environ.get("ATTN_PPB", "2"))
PV_OFF = 1             # delay P@V groups past the unit boundary
CAP_CYC = int(os.environ.get("ATTN_CAP", "2491"))   # PE cycles per exp block
CREDIT = int(os.environ.get("ATTN_CREDIT", "3000"))
FORCE_CAP = int(os.environ.get("ATTN_FORCE", "3100"))

# Unit order: 'stag' staggers pair starts (K/V deadlines) and chunk
# completions (o_proj releases) across the run.
_ORDER = os.environ.get("ATTN_ORDER", "stag")
if _ORDER == "pair":
    UNITS = [(qc, p) for p in range(4) for qc in range(4)]
elif _ORDER == "stag":
    UNITS = [
        (0, 0), (1, 0), (2, 0), (0, 1),
        (1, 1), (0, 2), (1, 2), (0, 3),
        (1, 3), (2, 1), (3, 0), (2, 2),
        (3, 1), (2, 3), (3, 2), (3, 3),
    ]
else:  # chunk-major
    UNITS = [(qc, p) for qc in range(4) for p in range(4)]
NBLK = len(UNITS) * KT  # 256

# Availability estimates (exp-block index when a DMA'd tensor lands),
# derived from the prologue DMA order below.
AVAIL_HT = {0: -1, 1: 0, 2: 2, 3: 6}
AVAIL_CS = {0: -1, 1: 0, 2: 3, 3: 3}
AVAIL_WQK = [-1, 12, 12, 12]   # per pair (pair 0 tiles load first)
AVAIL_WV = [0, 8, 8, 8]        # per pair (pair 0 loads first)

_CACHED_NC = None


def _build_nc():
    nc = bass.Bass()
    hTh = nc.declare_dram_parameter("hTh", [128, NSC, HT, SC], E4, isOutput=False)
    hTl = nc.declare_dram_parameter("hTl", [128, NSC, HT, SC], E4, isOutput=False)
    wqkh = nc.declare_dram_parameter("wqkh", [128, 2 * PAIRS, HT, 128], E4, isOutput=False)
    wqkl = nc.declare_dram_parameter("wqkl", [128, 2 * PAIRS, HT, 128], E4, isOutput=False)
    wvh = nc.declare_dram_parameter("wvh", [128, PAIRS, HT, 128], E4, isOutput=False)
    wvl = nc.declare_dram_parameter("wvl", [128, PAIRS, HT, 128], E4, isOutput=False)
    wo = nc.declare_dram_parameter("wo", [128, VF // 128, H], BF, isOutput=False)
    cossin = nc.declare_dram_parameter("cossin", [128, NSC, 2, SC], BF, isOutput=False)
    outT = nc.declare_dram_parameter("outT", [H, S], BF, isOutput=True)

    Exp = mybir.ActivationFunctionType.Exp
    MULT = mybir.AluOpType.mult
    DROW = mybir.MatmulPerfMode.DoubleRow

    with tile.TileContext(nc) as tc:
        with tc.tile_pool(name="singles", bufs=1) as singles:
            hTh_sb = singles.tile([128, NSC, HT, SC], E4)
            hTl_sb = singles.tile([128, NSC, HT, SC], E4)
            wqkh_sb = singles.tile([128, 2 * PAIRS, HT, 128], E4)
            wqkl_sb = singles.tile([128, 2 * PAIRS, HT, 128], E4)
            wvh_sb = singles.tile([128, PAIRS, HT, 128], E4)
            wvl_sb = singles.tile([128, PAIRS, HT, 128], E4)
            wo_sb = singles.tile([128, VF // 128, H], BF)
            cossin_sb = singles.tile([128, NSC, 2, SC], BF)
            q_rope = singles.tile([128, PAIRS, S], BF)
            k_rope = singles.tile([128, PAIRS, S], BF)
            vext = singles.tile([128, KT, HPC * 65], BF)
            nc.gpsimd.memset(vext[:], 1.0)  # ones columns for denominators

            # ---- DMA: per-chunk contiguous transfers, priority-ordered for
            # the prologue critical path (Q/K proj of (0, p0), then chunk 1,
            # then V weights, then the rest).
            dma = nc.sync.dma_start
            dma(out=hTh_sb[:, 0], in_=hTh[:, 0])
            dma(out=hTl_sb[:, 0], in_=hTl[:, 0])
            dma(out=wqkh_sb[:, 0:2], in_=wqkh[:, 0:2])
            dma(out=wqkl_sb[:, 0:2], in_=wqkl[:, 0:2])
            dma(out=cossin_sb[:, 0], in_=cossin[:, 0])
            dma(out=cossin_sb[:, 1], in_=cossin[:, 1])
            dma(out=hTh_sb[:, 1], in_=hTh[:, 1])
            dma(out=hTl_sb[:, 1], in_=hTl[:, 1])
            dma(out=wvh_sb[:, 0], in_=wvh[:, 0])
            dma(out=wvl_sb[:, 0], in_=wvl[:, 0])
            dma(out=hTh_sb[:, 2], in_=hTh[:, 2])
            dma(out=hTl_sb[:, 2], in_=hTl[:, 2])
            dma(out=cossin_sb[:, 2:4], in_=cossin[:, 2:4])
            dma(out=hTh_sb[:, 3], in_=hTh[:, 3])
            dma(out=hTl_sb[:, 3], in_=hTl[:, 3])
            dma(out=wvh_sb[:, 1:], in_=wvh[:, 1:])
            dma(out=wvl_sb[:, 1:], in_=wvl[:, 1:])
            dma(out=wqkh_sb[:, 2:], in_=wqkh[:, 2:])
            dma(out=wqkl_sb[:, 2:], in_=wqkl[:, 2:])
            dma(out=wo_sb[:], in_=wo[:])

            # ---- pools (PSUM: sps 4 + atps 2 + projps 2 = 8 banks) ----
            with (
                tc.tile_pool(name="sps", bufs=2, space="PSUM") as sps,
                tc.tile_pool(name="atps", bufs=2, space="PSUM") as atps,
                tc.tile_pool(name="projps", bufs=2, space="PSUM") as projps,
                tc.tile_pool(name="ropet", bufs=3) as ropet,
                tc.tile_pool(name="eps", bufs=EPS_BUFS) as eps,
                tc.tile_pool(name="anq", bufs=2) as anq,
                tc.tile_pool(name="recs", bufs=4) as recs,
                tc.tile_pool(name="ant", bufs=4) as ant,
                tc.tile_pool(name="obs", bufs=4) as obs,
            ):
                # PE warmup: the p-state model needs ~3us of continuous PE
                # execution to reach full clock.  Run throwaway matmuls while
                # the first DMAs land so the real stream starts warm.
                dummy_sb = singles.tile([128, SC], BF)
                nc.vector.memset(dummy_sb[:], 0.0)
                wps = projps.tile([128, SC], F32, tag="pj", name="wps")
                for _ in range(int(os.environ.get("ATTN_WARM", "14"))):
                    nc.tensor.matmul(
                        wps[0:1, :],
                        dummy_sb[:, 0:1],
                        dummy_sb[:],
                        start=True,
                        stop=True,
                    )

                pending_proj = {}
                JP = HT // 2  # DoubleRow tile-pairs in the contraction
                # 3-term hi/lo fp8: w_hi*h_hi + w_lo*h_hi + w_hi*h_lo (the
                # w_lo*h_lo term is ~0.4% of a term's scale and dropped).
                TERMS = [(0, 0), (1, 0), (0, 1)]  # (w lo?, h lo?)

                def proj_qk(m, c, phase, rope_split=1):
                    """Project q/k feature tile m (0-3 q pairs, 4-7 k pairs)
                    for seq chunk c; RoPE on DVE after the last phase.
                    12 DoubleRow matmuls split into 3 phases of 4."""
                    key = ("qk", m, c)
                    if phase == 0:
                        ps = projps.tile([128, SC], F32, tag="pj", name="pj")
                        pending_proj[key] = ps
                    else:
                        ps = pending_proj[key]
                    pair = m % PAIRS
                    ft = 2 * pair + (0 if m < PAIRS else 1)  # interleaved tile
                    mms = [
                        (wl, hl, j) for j in range(JP) for (wl, hl) in TERMS
                    ]
                    wtabs = (wqkh_sb, wqkl_sb)
                    htabs = (hTh_sb, hTl_sb)
                    for i in range(4 * phase, 4 * phase + 4):
                        wl, hl, j = mms[i]
                        nc.tensor.matmul(
                            ps[:],
                            wtabs[wl][:, ft, 2 * j : 2 * j + 2, :],
                            htabs[hl][:, c, 2 * j : 2 * j + 2, :],
                            start=(i == 0),
                            stop=(i == 11),
                            perf_mode=DROW,
                        )
                    if phase < 2:
                        return
                    del pending_proj[key]
                    dst_t = q_rope if m < PAIRS else k_rope
                    raw = ropet.tile([128, SC], BF, tag="raw")
                    t1 = ropet.tile([128, SC], BF, tag="t1")
                    t2 = ropet.tile([128, SC], BF, tag="t2")
                    w = SC // rope_split
                    for s in range(rope_split):
                        lo, hi = s * w, (s + 1) * w
                        nc.vector.tensor_copy(raw[:, lo:hi], ps[:, lo:hi])
                        cs = cossin_sb[:, c, 0, lo:hi]
                        sn = cossin_sb[:, c, 1, lo:hi]
                        dst = dst_t[:, pair, c * SC + lo : c * SC + hi]
                        nc.vector.tensor_mul(t1[:, lo:hi], raw[:, lo:hi], cs)
                        nc.vector.tensor_mul(t2[0:32, lo:hi], raw[32:64, lo:hi], sn[32:64])
                        nc.vector.tensor_mul(t2[32:64, lo:hi], raw[0:32, lo:hi], sn[0:32])
                        nc.vector.tensor_mul(t2[64:96, lo:hi], raw[96:128, lo:hi], sn[96:128])
                        nc.vector.tensor_mul(t2[96:128, lo:hi], raw[64:96, lo:hi], sn[64:96])
                        nc.vector.tensor_add(dst, t1[:, lo:hi], t2[:, lo:hi])

                def proj_v(st, p):
                    """Project V features of pair p for kpos block st.
                    12 DoubleRow matmuls (h stationary), single phase."""
                    ps = projps.tile([128, SC], F32, tag="pj", name="pj")
                    c_st, ko = st // 4, (st % 4) * 128
                    htabs = (hTh_sb, hTl_sb)
                    wtabs = (wvh_sb, wvl_sb)
                    i = 0
                    for j in range(JP):
                        for wl, hl in TERMS:
                            nc.tensor.matmul(
                                ps[:, 0:128],
                                htabs[hl][:, c_st, 2 * j : 2 * j + 2, ko : ko + 128],
                                wtabs[wl][:, p, 2 * j : 2 * j + 2, :],
                                start=(i == 0),
                                stop=(i == 11),
                                perf_mode=DROW,
                            )
                            i += 1
                    vdst = vext[:, st, :].rearrange(
                        "q (h x) -> q h x", x=65
                    )[:, 2 * p : 2 * p + 2, 0:64]
                    vsrc = ps[:, 0:128].rearrange("q (h x) -> q h x", x=64)
                    # psum holds 32x-scaled V (w' = 32w); descale on copy
                    nc.vector.tensor_scalar(vdst, vsrc, 1.0 / WSCALE, None, MULT)

                def emit_scores(g):
                    qc, pair = UNITS[g // KT]
                    kt = g % KT
                    ksl = slice(kt * 128, (kt + 1) * 128)
                    qsl = slice(qc * SC, (qc + 1) * SC)
                    s2 = sps.tile([128, 2, SC], F32, tag="s2", name="s2")
                    nc.tensor.matmul(
                        s2[:, 0, :],
                        k_rope[0:64, pair, ksl],
                        q_rope[0:64, pair, qsl],
                        start=True,
                        stop=True,
                        tile_position=(0, 0),
                    )
                    nc.tensor.matmul(
                        s2[:, 1, :],
                        k_rope[64:128, pair, ksl],
                        q_rope[64:128, pair, qsl],
                        start=True,
                        stop=True,
                        tile_position=(64, 0),
                    )
                    return s2

                def o_proj_m(qc, m, phase):
                    """o_proj feature tile m for chunk qc, 2 phases of 2 mms."""
                    key = ("op", qc, m)
                    qsl = slice(qc * SC, (qc + 1) * SC)
                    if phase == 0:
                        op = projps.tile([128, SC], F32, tag="pj", name="pj")
                        pending_proj[key] = op
                    else:
                        op = pending_proj[key]
                    at = attnT_of[qc]
                    for ot in range(2 * phase, 2 * phase + 2):
                        nc.tensor.matmul(
                            op[:],
                            wo_sb[:, ot, m * 128 : (m + 1) * 128],
                            at[:, ot, :],
                            start=(ot == 0),
                            stop=(ot == VF // 128 - 1),
                        )
                    if phase == 0:
                        return
                    del pending_proj[key]
                    ob = obs.tile([128, SC], BF, tag="ob", name="ob")
                    nc.vector.tensor_copy(ob[:], op[:])
                    nc.sync.dma_start(out=outT[m * 128 : (m + 1) * 128, qsl], in_=ob[:])

                # ---------- P@V groups, normalize, transpose ----------
                e_of = {}       # g -> e tile AP
                s2_of = {}      # g -> scores psum AP
                aq_of = {}      # unit idx -> attn_q sbuf AP
                attnT_of = {}   # qc -> attnT sbuf AP

                # group order: both halves of a q-block back-to-back, so
                # its transpose DMA can fire while later groups still run
                GRP_ORDER = [0, 4, 1, 5, 2, 6, 3, 7]

                def emit_pv_group(ui, gi, att_ap=None, tq=None):
                    """One P@V output group: accumulate [128 q, 65] over all
                    kt for (half, qb) = divmod(grp, QB), then normalize."""
                    grp = GRP_ORDER[gi]
                    qc, pair = UNITS[ui]
                    half, qb = divmod(grp, QB)
                    hloc = 2 * pair + half
                    if att_ap is None:
                        att = atps.tile([128, 65], F32, tag="att", name="att")
                    else:
                        att = att_ap
                    for kt in range(KT):
                        est = e_of[ui * KT + kt][:, half, qb * 128 : (qb + 1) * 128]
                        vsl = vext[:, kt, hloc * 65 : (hloc + 1) * 65]
                        nc.tensor.matmul(
                            att[:],
                            est,
                            vsl,
                            start=(kt == 0),
                            stop=(kt == KT - 1),
                        )
                    rec = recs.tile([128, 1], F32, tag="rec", name="rec")
                    nc.vector.reciprocal(rec[:], att[:, 64:65])
                    if gi == 0:
                        aq_of[ui] = anq.tile(
                            [128, QB, 128], BF, tag="aq", name="aq"
                        )
                    aq = aq_of[ui]
                    nc.vector.tensor_scalar(
                        aq[:, qb, half * 64 : (half + 1) * 64],
                        att[:, 0:64],
                        rec[:, 0:1],
                        None,
                        MULT,
                    )
                    if half == 1:
                        # both halves of this q-block done: transpose it now
                        if qc not in attnT_of:
                            attnT_of[qc] = ant.tile(
                                [128, PAIRS, SC], BF, tag="at", name="at"
                            )
                        nc.sync.dma_start_transpose(
                            attnT_of[qc][:, pair, qb * 128 : (qb + 1) * 128],
                            aq[:, qb, :],
                        )
                    if gi == 7:
                        for g in range(ui * KT, (ui + 1) * KT):
                            del e_of[g]

                # ---------- background work items ----------
                first_blk_of_pair = {}
                last_unit_of_chunk = {}
                for i, (qc, p) in enumerate(UNITS):
                    first_blk_of_pair.setdefault(p, i * KT)
                    last_unit_of_chunk[qc] = i
                unit_start = {u: i * KT for i, u in enumerate(UNITS)}

                # item: (dl, seq, avail, kind, args, [phase cycle costs])
                items = []
                seq = 0
                QK_CYC = [1024, 1024, 1024]
                for p in range(PAIRS):
                    for c in range(NSC):
                        dl = first_blk_of_pair[p] + 4 * c - 8
                        avail = max(AVAIL_HT[c], AVAIL_CS[c], AVAIL_WQK[p])
                        items.append((dl, seq, avail, "K", (p, c), QK_CYC))
                        seq += 1
                for (qc, p), st_blk in unit_start.items():
                    dl = st_blk - 8
                    avail = max(AVAIL_HT[qc], AVAIL_CS[qc], AVAIL_WQK[p])
                    items.append((dl, seq, avail, "Q", (qc, p), QK_CYC))
                    seq += 1
                for p in range(PAIRS):
                    for st in range(KT):
                        dl = first_blk_of_pair[p] + 7 + st // 2
                        avail = max(AVAIL_HT[st // 4], AVAIL_WV[p])
                        items.append((dl, seq, avail, "V", (st, p), [768]))
                        seq += 1
                last_ui = len(UNITS) - 1
                qc_last = UNITS[last_ui][0]
                for qc in range(NSC):
                    if qc == qc_last:
                        continue  # emitted by the special tail path
                    lu = last_unit_of_chunk[qc]
                    rel = lu * KT + KT + PV_OFF + (8 + PV_PER_BLOCK - 1) // PV_PER_BLOCK + 1
                    for m in range(HT):
                        items.append((rel + 2 + 3 * m, seq, rel, "OP", (qc, m), [1024, 1024]))
                        seq += 1

                def emit_item_phase(kind, args, phase):
                    if kind == "K":
                        p, c = args
                        rs = {(0, 0): 4, (0, 1): 2}.get((p, c), 1)
                        proj_qk(PAIRS + p, c, phase, rope_split=rs)
                    elif kind == "Q":
                        qc, p = args
                        proj_qk(p, qc, phase)
                    elif kind == "V":
                        proj_v(*args)
                    elif kind == "OP":
                        qc, m = args
                        o_proj_m(qc, m, phase)

                # P@V group schedule; the final unit is handled by the
                # custom tail sequence below.
                pv_by_block = {}
                for ui in range(len(UNITS) - 1):
                    for i in range(8):
                        blk = ui * KT + KT + PV_OFF + i // PV_PER_BLOCK
                        pv_by_block.setdefault(blk, []).append((ui, i))

                # ---------- prologue ----------
                avail_at = {}
                heap = []
                for dl, sq, avail, kind, args, cyc in items:
                    if avail < 0:
                        continue  # prologue items handled below
                    avail_at.setdefault(avail, []).append((dl, sq, kind, args, cyc))
                prolog = [it for it in items if it[2] < 0]
                # Q first: scores(0) needs the full q_rope chunk, while only
                # the first quarter of the K chunk gates it
                prolog.sort(key=lambda t: (t[3] != "Q", t[3] != "K", t[1]))
                for dl, sq, avail, kind, args, cyc in prolog:
                    for ph in range(len(cyc)):
                        emit_item_phase(kind, args, ph)
                s2_of[0] = emit_scores(0)
                s2_of[1] = emit_scores(1)

                # ---------- main loop: greedy EDF with PE cycle budget ----
                cum = 0
                open_item = None  # (dl, sq, kind, args, cyc, next_phase)
                NTOT = NBLK + 8
                for g in range(NTOT):
                    for it in avail_at.pop(g, []):
                        heapq.heappush(heap, it)
                    if g + 2 < NBLK:
                        s2_of[g + 2] = emit_scores(g + 2)
                        cum += 1024
                    if g < NBLK:
                        s2 = s2_of.pop(g)
                        e = eps.tile([128, 2, SC], BF)
                        nc.scalar.activation(
                            out=e[:], in_=s2[:], func=Exp, scale=EXP_SCALE
                        )
                        e_of[g] = e
                    # reserve cycles for this block's P@V groups
                    pv_list = pv_by_block.get(g, ())
                    cum += 1040 * len(pv_list)
                    budget = CREDIT + (min(g, NBLK - 1) + 1) * CAP_CYC - cum
                    forced = 0
                    while open_item is not None or heap:
                        overdue = (
                            g >= NBLK
                            or open_item is not None
                            or (heap and heap[0][0] <= g)
                        )
                        if budget <= 0 and not overdue:
                            break
                        if budget <= 0 and open_item is None and forced >= FORCE_CAP:
                            break  # defer further overdue items to next block
                        if open_item is None:
                            dl, sq, kind, args, cyc = heapq.heappop(heap)
                            open_item = [dl, sq, kind, args, cyc, 0]
                        dl, sq, kind, args, cyc, ph = open_item
                        emit_item_phase(kind, args, ph)
                        cum += cyc[ph]
                        if budget <= 0:
                            forced += cyc[ph]
                        budget -= cyc[ph]
                        if ph + 1 < len(cyc):
                            open_item[5] = ph + 1
                        else:
                            open_item = None
                    for ui, grp in pv_list:
                        emit_pv_group(ui, grp)

                # ---------- tail: last unit's P@V + last chunk's o_proj ----
                # The scores psum is idle now (no more scores): borrow its 4
                # banks -- 2 widen the P@V group ring to 4-way (halves the
                # ping-pong serialization), 2 host extra open o_proj groups.
                # o_proj ot 0-2 read pairs 0-2 (transposed long ago) and
                # interleave with the P@V quads; the ot-3 matmuls are
                # emitted qb-sliced so each waits only its own transpose.
                qsl = slice(qc_last * SC, (qc_last + 1) * SC)
                at_last = attnT_of[qc_last]
                sps_pv = sps.tile([128, 2, SC], F32, tag="s2", name="s2")
                sps_op = sps.tile([128, 2, SC], F32, tag="s2", name="s2")
                ps_of = {}

                def tail_p0(m, slot):
                    if slot is None:
                        ps = projps.tile([128, SC], F32, tag="pj", name="pj")
                    else:
                        ps = sps_op[:, slot, :]
                    ps_of[m] = ps
                    for ot in range(3):
                        nc.tensor.matmul(
                            ps[:],
                            wo_sb[:, ot, m * 128 : (m + 1) * 128],
                            at_last[:, ot, :],
                            start=(ot == 0),
                            stop=False,
                        )

                def tail_p1(m):
                    ps = ps_of.pop(m)
                    for qb in range(QB):
                        cs = slice(qb * 128, (qb + 1) * 128)
                        nc.tensor.matmul(
                            ps[:, cs],
                            wo_sb[:, 3, m * 128 : (m + 1) * 128],
                            at_last[:, 3, cs],
                            start=False,
                            stop=(qb == QB - 1),
                        )
                    ob = obs.tile([128, SC], BF, tag="ob", name="ob")
                    nc.vector.tensor_copy(ob[:], ps[:])
                    nc.sync.dma_start(
                        out=outT[m * 128 : (m + 1) * 128, qsl], in_=ob[:]
                    )

                # P@V quad 1 (qb 0-1), o_proj p0s, quad 2 (qb 2-3)
                emit_pv_group(last_ui, 0)
                emit_pv_group(last_ui, 1)
                emit_pv_group(last_ui, 2, att_ap=sps_pv[:, 0, 0:65])
                emit_pv_group(last_ui, 3, att_ap=sps_pv[:, 1, 0:65])
                tail_p0(0, None)
                tail_p0(1, None)
                tail_p0(2, 0)
                tail_p0(3, 1)
                emit_pv_group(last_ui, 4)
                emit_pv_group(last_ui, 5)
                emit_pv_group(last_ui, 6, att_ap=sps_pv[:, 0, 0:65])
                emit_pv_group(last_ui, 7, att_ap=sps_pv[:, 1, 0:65])
                for m in range(4):
                    tail_p1(m)
                tail_p0(4, None)
                tail_p0(5, None)
                tail_p0(6, 0)
                tail_p0(7, 1)
                for m in range(4, 8):
                    tail_p1(m)
    _split_excess_waits(nc)
    return nc


def _hilo(x):
    """fp8e4m3 hi/lo split: x ~= hi + lo (f32 in, two fp8 arrays out)."""
    hi = x.astype(E4NP)
    lo = (x - hi.astype(np.float32)).astype(E4NP)
    return hi, lo


def _prep_inputs(cos, sin, hidden_states, w_qkv, w_o):
    """Per-core host-side sharding/transpose/cast. Returns list of in_maps."""
    bf = np.float16
    cos = np.asarray(cos, np.float32)
    sin = np.asarray(sin, np.float32)
    hidden_states = np.asarray(hidden_states, np.float32)
    w_qkv = np.asarray(w_qkv, np.float32) * WSCALE
    w_o = np.asarray(w_o, np.float32)

    # cos/sin tables carry the 1/WSCALE descale of the 32x-scaled q/k psums
    cosT = cos.T / WSCALE  # [64, S]
    cos_t = np.ascontiguousarray(np.tile(cosT, (2, 1))).astype(bf)
    # sin multiplier aligned to the *source* partitions of the rot ops:
    # rows [0:32] = +sin[32:64] (multiplies src q[0:32] -> dest [32:64]),
    # rows [32:64] = -sin[0:32] (multiplies src q[32:64] -> dest [0:32]).
    sinT = sin.T / WSCALE
    sin_t = np.ascontiguousarray(
        np.tile(np.concatenate([sinT[32:], -sinT[:32]], 0), (2, 1))
    ).astype(bf)
    # [128, NSC, 2, SC] chunk-major
    cos4 = cos_t.reshape(128, NSC, SC)
    sin4 = sin_t.reshape(128, NSC, SC)
    cossin = np.ascontiguousarray(np.stack([cos4, sin4], axis=2))

    def feat_tiles(w_t, order):
        # w_t: [128, HT, F] -> [128, len(order), HT, 128] per feature tile
        return np.ascontiguousarray(
            np.stack([w_t[:, :, 128 * j : 128 * (j + 1)] for j in order], axis=1)
        )

    in_maps = []
    for core in range(8):
        b, g = core // 2, core % 2
        hT = hidden_states[b].T  # [H, S]
        hTh, hTl = _hilo(hT)

        def chunkmajor(x):  # [H, S] fp8 -> [128, NSC, HT, SC]
            return np.ascontiguousarray(
                x.reshape(HT, 128, NSC, SC).transpose(1, 2, 0, 3)
            )

        qs, ks, vs = g * VF, NH * HD + g * VF, 2 * NH * HD + g * VF
        wqk_rows = np.concatenate(
            [w_qkv[qs : qs + VF], w_qkv[ks : ks + VF]], 0
        )  # [QKF, H]
        wqkh, wqkl = _hilo(wqk_rows)
        # feature-tile-major, interleaved [q0, k0, q1, k1, ...]
        interleave = [j for p in range(PAIRS) for j in (p, PAIRS + p)]

        def wqk_fmt(w):
            w_t = w.T.reshape(HT, 128, QKF).transpose(1, 0, 2)  # [128, HT, QKF]
            return feat_tiles(w_t, interleave)

        wvh_, wvl_ = _hilo(w_qkv[vs : vs + VF])

        def wv_fmt(w):
            w_t = w.T.reshape(HT, 128, VF).transpose(1, 0, 2)
            return feat_tiles(w_t, range(PAIRS))

        woT = w_o[:, g * VF : (g + 1) * VF].T  # [VF, H]
        wo_t = np.ascontiguousarray(
            woT.reshape(VF // 128, 128, H).transpose(1, 0, 2)
        ).astype(bf)
        in_maps.append(
            {
                "hTh": chunkmajor(hTh),
                "hTl": chunkmajor(hTl),
                "wqkh": wqk_fmt(wqkh),
                "wqkl": wqk_fmt(wqkl),
                "wvh": wv_fmt(wvh_),
                "wvl": wv_fmt(wvl_),
                "wo": wo_t,
                "cossin": cossin,
            }
        )
    return in_maps


def kernel(cos, sin, hidden_states, w_qkv, w_o, _trace=False):
    global _CACHED_NC
    if _CACHED_NC is None:
        _CACHED_NC = _build_nc()
    nc = _CACHED_NC
    in_maps = _prep_inputs(cos, sin, hidden_states, w_qkv, w_o)
    res = run_bass_kernel_spmd(nc, in_maps, core_ids=list(range(8)), trace=_trace)
    outs = [r["outT"] for r in res.results]
    out = np.empty((B, S, H), np.float32)
    for b in range(B):
        out[b] = (
            outs[2 * b].astype(np.float32) + outs[2 * b + 1].astype(np.float32)
        ).T
    if _trace:
        return out, res
    return out
